# revision 14
# baseline (speedup 1.0000x reference)
"""LCA layer kernel for Trainium2, data-parallel over tokens on 8 NeuronCores.

Reference computation (per token row x of d_model=1024, W [1024, 4096]):
    b = x @ W;  G = W^T W with zero diag;  u_0 = 0
    10x: a = relu(u - 0.1); u = 0.9 u + 0.1 (b - a @ G)
    out = relu(u - 0.1) @ W^T

Device algorithm (per core, 1024 tokens = 4 blocks of 256, feature-major
[d_lca, token] state layout):
  * factor a@G = (a @ W^T) @ W - g * a  with g = diag(W^T W), halving FLOPs.
  * u after step 1 is exactly B' = 0.1 b (u0=0 -> a0=0), so 9 iterated steps.
  * the two iterated matmuls (90% of FLOPs) run in fp8e4 with
    perf_mode=DoubleRow (2 fp8 MACs/PE-cell/cycle, K=256 per pass):
        ht8 = Sh * (W a^T)        from a8 = Sa*relu(u-lam)  [fp8]
        py  = Sw*Sh * (W^T W a^T) from ht8 and w018 = Sw*W  [fp8]
  * the py PSUM group folds in ALL linear update terms, so PSUM = K*u'
    with K = 1/c1 = -Sw*Sh/0.1 = -40960:
      - kbp = K*B' and us = 0.9K*u' (both bf16) live interleaved in one
        state tile, so a single 512-column identity matmul adds both
      - the diagonal 0.1*g*a term rides as a 5th DoubleRow pair
        (gdiag[jc], 0) x (a8[jc], 0) against zero-padded tiles
      - every DVE_INIT_EVERY-th chunk skips the two identity matmuls;
        a single DVE stt writes the PSUM init (0.9K*u + kbp) instead,
        balancing PE vs DVE occupancy
    extraction per chunk: one DVE stt (u = c1*py -> bf16) and one ACT
    relu (a8 = relu(Sa*c1*py - Sa*lam) -> fp8) straight from PSUM.
  * B' (= u init) and out = relu(u-lam) @ W^T stay bf16 for accuracy; their
    bf16 weights are streamed from HBM in chunks (double-buffered) instead of
    held in SBUF, so the resident weights are only the two fp8 copies.
  * emulated end-to-end error vs fp32 reference: rel_l2 ~ 8.5e-3 (gate 2e-2).
"""

import numpy as np
import ml_dtypes

P = 128          # partitions
T = 256          # tokens per block
NBLK = 4         # blocks per core (4*256 = 1024 tokens/core)
NSTEPS = 9       # iterated steps (step 1 is the free u=B' init)
DM = 1024        # d_model
DL = 4096        # d_lca
NDM = DM // P    # 8 d_model chunks
NDL = DL // P    # 32 d_lca chunks
NCORES = 8
TOK_CORE = NBLK * T

SW = 128.0       # fp8 scale on W (both fp8 copies)
SA = 64.0        # fp8 scale on a
SH = 32.0        # fp8 scale on ht
LAM = 0.1
C1 = -0.1 / (SW * SH)
KC = 1.0 / C1    # -40960, bf16-exact
DVE_INIT_EVERY = 1000  # disabled: DVE PSUM-init gates the MM group (slower)

BF16 = ml_dtypes.bfloat16
E4M3 = ml_dtypes.float8_e4m3

_CACHE = {}

TRACE = False
LAST_RESULT = None


def _build_nc(nblk=None, nsteps=None):
    import concourse.bacc as bacc
    import concourse.tile as tile
    import concourse.mybir as mybir

    nblk = NBLK if nblk is None else nblk
    nsteps = NSTEPS if nsteps is None else nsteps
    tok_core = nblk * T

    dt = mybir.dt
    Alu = mybir.AluOpType
    Act = mybir.ActivationFunctionType
    DR = mybir.MatmulPerfMode.DoubleRow

    nc = bacc.Bacc("TRN2", target_bir_lowering=False, debug=False,
                   num_devices=NCORES)

    xt_d = nc.dram_tensor("xt", [nblk, P, NDM, T], dt.bfloat16,
                          kind="ExternalInput").ap()
    wt8_d = nc.dram_tensor("wt8", [P, NDL, DM], dt.float8e4,
                           kind="ExternalInput").ap()
    w018_d = nc.dram_tensor("w018", [P, NDM, DL], dt.float8e4,
                            kind="ExternalInput").ap()
    w01b_d = nc.dram_tensor("w01b", [NDM, P, NDM, 512], dt.bfloat16,
                            kind="ExternalInput").ap()
    wtb_d = nc.dram_tensor("wtb", [NDL, P, DM], dt.bfloat16,
                           kind="ExternalInput").ap()
    id1_d = nc.dram_tensor("id1", [P, P], dt.bfloat16,
                           kind="ExternalInput").ap()
    gdiagz_d = nc.dram_tensor("gdiagz", [P, NDL + 1, P], dt.float8e4,
                              kind="ExternalInput").ap()
    out_d = nc.dram_tensor("out", [tok_core, DM], dt.float32,
                           kind="ExternalOutput").ap()

    hscale = SH / (SW * SA)
    K09 = float(np.float32(0.9) * np.float32(KC))  # -36864

    with tile.TileContext(nc) as tc:
        with (
            tc.tile_pool(name="wpool", bufs=1) as wpool,
            tc.tile_pool(name="state", bufs=1) as state,
            tc.tile_pool(name="htp", bufs=1) as htp,
            tc.tile_pool(name="xio", bufs=2) as xio,
            tc.tile_pool(name="wstr", bufs=3) as wstr,
            tc.tile_pool(name="wstr2", bufs=3) as wstr2,
            tc.tile_pool(name="oio", bufs=2) as oio,
            tc.tile_pool(name="psum", bufs=4, space="PSUM") as psum,
            tc.tile_pool(name="psob", bufs=4, space="PSUM") as psob,
        ):
            # ---- resident fp8 weights + constants ----
            wt8 = wpool.tile([P, NDL, DM], dt.float8e4, tag="wt8")
            w018 = wpool.tile([P, NDM, DL], dt.float8e4, tag="w018")
            id1 = wpool.tile([P, P], dt.bfloat16, tag="id1")
            gdiagz = wpool.tile([P, NDL + 1, P], dt.float8e4, tag="gdiagz")
            nbA = wpool.tile([P, 1], dt.float32, tag="nbA")
            nbL = wpool.tile([P, 1], dt.float32, tag="nbL")
            zero = wpool.tile([P, T], dt.float32, tag="zero")
            nc.gpsimd.memset(nbA[:], -SA * LAM)
            nc.gpsimd.memset(nbL[:], -LAM)
            nc.gpsimd.memset(zero[:], 0.0)
            for kg in range(0, NDL, 8):
                nc.sync.dma_start(wt8[:, kg:kg + 8, :], wt8_d[:, kg:kg + 8, :])
            for dmc in range(NDM):
                nc.sync.dma_start(w018[:, dmc, :], w018_d[:, dmc, :])
            nc.sync.dma_start(id1[:], id1_d[:])
            nc.sync.dma_start(gdiagz[:], gdiagz_d[:])

            for blk in range(nblk):
                xt = xio.tile([P, NDM, T], dt.bfloat16, tag="xt")
                nc.sync.dma_start(xt[:], xt_d[blk])

                # kus[:, jc, 0, :] = K*B', kus[:, jc, 1, :] = 0.9K*u
                kus = state.tile([P, NDL, 2, T], dt.bfloat16, tag="kus")
                a8z = state.tile([P, NDL + 1, T], dt.float8e4, tag="a8z")
                nc.gpsimd.memset(a8z[:, NDL, :], 0.0)

                # ---- B' = x @ (0.1 W): u init, bf16 weights streamed ----
                for grp in range(NDM):
                    wg = wstr.tile([P, NDM, 512], dt.bfloat16, tag="wg")
                    nc.sync.dma_start(wg[:], w01b_d[grp])
                    for s4 in range(4):
                        jc = grp * 4 + s4
                        pb = psum.tile([P, T], dt.float32, tag="mm")
                        for dmc in range(NDM):
                            nc.tensor.matmul(
                                pb[:], wg[:, dmc, s4 * P:(s4 + 1) * P],
                                xt[:, dmc, :],
                                start=(dmc == 0), stop=(dmc == NDM - 1))
                        # kbp = K*B', us = 0.9K*B', a8 -- all from PSUM
                        nc.scalar.mul(kus[:, jc, 0, :], pb[:], KC)
                        nc.vector.scalar_tensor_tensor(
                            kus[:, jc, 1, :], pb[:], K09, zero[:],
                            op0=Alu.mult, op1=Alu.add)
                        nc.scalar.activation(a8z[:, jc, :], pb[:], Act.Relu,
                                             bias=nbA[:, 0:1], scale=SA)

                # ---- 9 iterated fp8 DoubleRow steps ----
                for _ in range(nsteps):
                    ht8 = htp.tile([P, NDM, T], dt.float8e4, tag="ht8")
                    for dmc in range(NDM):
                        ph = psum.tile([P, T], dt.float32, tag="mm")
                        for pc in range(NDL // 2):
                            nc.tensor.matmul(
                                ph[:],
                                wt8[:, 2 * pc:2 * pc + 2,
                                    dmc * P:(dmc + 1) * P],
                                a8z[:, 2 * pc:2 * pc + 2, :],
                                start=(pc == 0), stop=(pc == NDL // 2 - 1),
                                perf_mode=DR)
                        nc.scalar.mul(ht8[:, dmc, :], ph[:], hscale)

                    for jc in range(NDL):
                        py = psum.tile([P, T], dt.float32, tag="mm")
                        # identity matmuls add K*bp and 0.9K*u (shared
                        # stationary since us is stored pre-scaled)
                        nc.tensor.matmul(py[:], id1[:], kus[:, jc, 0, :],
                                         start=True, stop=False)
                        nc.tensor.matmul(py[:], id1[:], kus[:, jc, 1, :],
                                         start=False, stop=False)
                        # diag 0.1*g*a rides as a 5th DoubleRow pair
                        # against zero-padded partners
                        st = NDL - jc
                        nc.tensor.matmul(
                            py[:], gdiagz[:, jc:NDL + 1:st, :],
                            a8z[:, jc:NDL + 1:st, :],
                            start=False, stop=False, perf_mode=DR)
                        for pc in range(NDM // 2):
                            nc.tensor.matmul(
                                py[:],
                                w018[:, 2 * pc:2 * pc + 2,
                                     jc * P:(jc + 1) * P],
                                ht8[:, 2 * pc:2 * pc + 2, :],
                                start=False, stop=(pc == NDM // 2 - 1),
                                perf_mode=DR)
                        # PSUM = K*u' -> us = 0.9*py (bf16), a8 from PSUM
                        nc.vector.scalar_tensor_tensor(
                            kus[:, jc, 1, :], py[:], 0.9, zero[:],
                            op0=Alu.mult, op1=Alu.add)
                        nc.scalar.activation(a8z[:, jc, :], py[:], Act.Relu,
                                             bias=nbA[:, 0:1],
                                             scale=SA * C1)

                # ---- out = relu(u - lam) @ W^T in bf16, streamed weights ----
                a16 = htp.tile([P, NDL, T], dt.bfloat16, tag="a16")
                inv09k = 1.0 / (0.9 * KC)
                for jc in range(NDL):
                    nc.scalar.activation(a16[:, jc, :], kus[:, jc, 1, :],
                                         Act.Relu, bias=nbL[:, 0:1],
                                         scale=inv09k)
                po = [psob.tile([P, 512], dt.float32, tag="ob",
                                name=f"po{i}")
                      for i in range(4)]
                for kc in range(NDL):
                    wb = wstr2.tile([P, DM], dt.bfloat16, tag="wb")
                    nc.sync.dma_start(wb[:], wtb_d[kc])
                    for sub in range(2):
                        for nh in range(2):
                            nc.tensor.matmul(
                                po[sub * 2 + nh][:],
                                a16[:, kc, sub * P:(sub + 1) * P],
                                wb[:, nh * 512:(nh + 1) * 512],
                                start=(kc == 0), stop=(kc == NDL - 1),
                                skip_group_check=True)
                for sub in range(2):
                    ob = oio.tile([P, DM], dt.float32, tag="ob")
                    for nh in range(2):
                        nc.scalar.copy(ob[:, nh * 512:(nh + 1) * 512],
                                       po[sub * 2 + nh][:])
                    row = (blk * 2 + sub) * P
                    nc.sync.dma_start(out_d[row:row + P, :], ob[:])

    nc.compile()
    return nc


def _get_nc():
    if "nc" not in _CACHE:
        _CACHE["nc"] = _build_nc()
    return _CACHE["nc"]


def _prep_shared(W):
    W = np.asarray(W, np.float32)

    def q8(v, s):
        return np.clip(v * s, -240, 240).astype(E4M3)

    wt8 = np.ascontiguousarray(
        q8(W.T, SW).reshape(NDL, P, DM).transpose(1, 0, 2))
    w018 = np.ascontiguousarray(
        q8(W, SW).reshape(NDM, P, DL).transpose(1, 0, 2))
    # w01b[grp, p, dmc, c] = 0.1*W[dmc*128+p, grp*512+c]
    w01b = np.ascontiguousarray(
        (0.1 * W).astype(BF16).reshape(NDM, P, NDM, 512).transpose(2, 1, 0, 3))
    # wtb[kc, p, m] = W[m, kc*128+p]
    wtb = np.ascontiguousarray(
        W.T.astype(BF16).reshape(NDL, P, DM))
    g = (W.astype(np.float64) ** 2).sum(0)
    id1 = np.eye(P).astype(BF16)
    # gdiagz[p, jc, c] = (c==p) * 0.1*g[jc*128+p]*K/SA, zero pad chunk at end
    gd = (0.1 * g * KC / SA).reshape(NDL, P).T.astype(np.float32)  # [P, NDL]
    gdiagz = np.zeros((P, NDL + 1, P), np.float32)
    for pp in range(P):
        gdiagz[pp, :NDL, pp] = gd[pp, :]
    gdiagz = np.clip(gdiagz, -240, 240).astype(E4M3)
    return wt8, w018, w01b, wtb, id1, gdiagz


def _make_xt(xs):
    # xs [tok_core, DM] f32 -> [nblk, P, NDM, T] bf16
    nblk = xs.shape[0] // T
    return np.ascontiguousarray(
        xs.reshape(nblk, T, NDM, P).transpose(0, 3, 2, 1)).astype(BF16)


def bench_setup(reduced=False):
    """For bench.py: build the module and random same-shape inputs."""
    import os
    os.environ.setdefault("BASS_NEVER_TRACE", "1")
    if reduced:
        nc = _build_nc(nblk=1, nsteps=1)
        nblk = 1
    else:
        nc = _get_nc()
        nblk = NBLK
    rng = np.random.default_rng(0)
    W = (rng.standard_normal((DM, DL)) * 0.02).astype(np.float32)
    wt8, w018, w01b, wtb, id1, gdiagz = _prep_shared(W)
    in_maps = []
    for c in range(NCORES):
        xs = rng.standard_normal((nblk * T, DM)).astype(np.float32)
        in_maps.append({"xt": _make_xt(xs), "wt8": wt8, "w018": w018,
                        "w01b": w01b, "wtb": wtb, "id1": id1,
                        "gdiagz": gdiagz})
    return nc, in_maps


def kernel(x, W):
    import os

    from concourse.bass_utils import run_bass_kernel_spmd

    if not TRACE:
        os.environ.setdefault("BASS_NEVER_TRACE", "1")
    x = np.asarray(x)
    orig_shape = x.shape
    xf = x.reshape(-1, DM).astype(np.float32)
    wt8, w018, w01b, wtb, id1, gdiagz = _prep_shared(W)

    in_maps = []
    for c in range(NCORES):
        xs = xf[c * TOK_CORE:(c + 1) * TOK_CORE]
        in_maps.append({"xt": _make_xt(xs), "wt8": wt8, "w018": w018,
                        "w01b": w01b, "wtb": wtb, "id1": id1,
                        "gdiagz": gdiagz})

    nc = _get_nc()
    res = run_bass_kernel_spmd(nc, in_maps, core_ids=list(range(NCORES)),
                               trace=TRACE)
    global LAST_RESULT
    LAST_RESULT = res
    out = np.concatenate([res.results[c]["out"] for c in range(NCORES)],
                         axis=0)
    return out.reshape(orig_shape).astype(np.float32)
